# revision 2
# baseline (speedup 1.0000x reference)
"""DiagonalAffine kernel for Trainium2: y = x * A_diag + B.

x: (262144, 512) f32. Data-parallel over 8 NeuronCores: each core gets a
contiguous slice of 32768 rows.

Per-core design (derived from NTFF trace analysis of the f32 baseline):
the 16 SDMA engines aggregate ~424 GB/s one-way and were 95% busy, and the
DVE (0.96 GHz, 1 f32 elem/cycle on tensor_tensor) was 85% busy doing
2 ops/element. Two levers:

1. Store the output as bf16 (final rounding error <= 2^-8 relative to each
   element -- well inside the 2e-2 gate; the f32 mul/add path is bit-exact
   vs the reference). Traffic drops 128MiB -> 96MiB per core.
2. Balance the DMA queues so all three drain together: loads are split by
   partition half across the two HWDGE rings (sync: partitions 0-63 = even
   SDMA engines, scalar: 64-127 = odd engines), stores ride the SWDGE
   (gpsimd) queue with 8KB partition lines vs the loads' 16KB lines --
   each engine sees 2:1 load:store bytes per round-robin cycle, matching
   the 2:1 byte ratio of the streams.

Compute: DVE does every f32 multiply (bit-exactness: a mul deviation
scales with |x*a| and would blow the elementwise rel check at cancellation
points; an add deviation scales with |y| and is safe) plus half the adds;
gpsimd (2x slower per element) does the other half of the adds and the
store descriptor generation. Adds write bf16 tiles directly.
"""

import os
import sys

import numpy as np

_TRN_REPO = "/opt/trn_rl_repo"
if os.path.isdir(_TRN_REPO) and _TRN_REPO not in sys.path:
    sys.path.insert(0, _TRN_REPO)

N, D = 262144, 512
N_CORES = 8
ROWS_PER_CORE = N // N_CORES  # 32768

P = 128              # SBUF partitions
F_ROWS = int(os.environ.get("K_F_ROWS", "8"))   # rows of x per partition per tile
TILE_FREE = F_ROWS * D
ROWS_PER_TILE = P * F_ROWS                      # 1024
X_BUFS = int(os.environ.get("K_XBUFS", "7"))
Y_BUFS = int(os.environ.get("K_YBUFS", "7"))
# every GP_EVERY-th tile's add runs on gpsimd (0 = never)
GP_EVERY = int(os.environ.get("K_GP_EVERY", "2"))

_BUILD_CACHE: dict = {}


def _build(rows_per_core: int):
    """Build the per-core Bass program (identical on all cores)."""
    import concourse.bacc as bacc
    import concourse.tile as tile
    from concourse import mybir

    f32 = mybir.dt.float32
    bf16 = mybir.dt.bfloat16
    n_tiles = rows_per_core // ROWS_PER_TILE
    assert n_tiles * ROWS_PER_TILE == rows_per_core

    nc = bacc.Bacc("TRN2", debug=False, num_devices=N_CORES)
    x_in = nc.dram_tensor("x", [rows_per_core, D], f32, kind="ExternalInput")
    a_in = nc.dram_tensor("a_rep", [P, D], f32, kind="ExternalInput")
    b_in = nc.dram_tensor("b_rep", [P, D], f32, kind="ExternalInput")
    y_out = nc.dram_tensor("y", [rows_per_core, D], bf16, kind="ExternalOutput")

    xv = x_in[:, :].rearrange("(t p f) d -> t p (f d)", p=P, f=F_ROWS)
    yv = y_out[:, :].rearrange("(t p f) d -> t p (f d)", p=P, f=F_ROWS)
    HALF = P // 2

    with tile.TileContext(nc) as tc:
        with (
            tc.tile_pool(name="const", bufs=1) as cpool,
            tc.tile_pool(name="xp", bufs=X_BUFS) as xpool,
            tc.tile_pool(name="yp", bufs=Y_BUFS) as ypool,
        ):
            a_t = cpool.tile([P, D], f32, tag="a")
            nc.sync.dma_start(out=a_t[:], in_=a_in[:, :])
            b_t = cpool.tile([P, D], f32, tag="b")
            nc.scalar.dma_start(out=b_t[:], in_=b_in[:, :])

            a_ap = a_t[:, :].unsqueeze(1).to_broadcast((P, F_ROWS, D))
            b_ap = b_t[:, :].unsqueeze(1).to_broadcast((P, F_ROWS, D))

            for t in range(n_tiles):
                xt = xpool.tile([P, TILE_FREE], f32)
                nc.sync.dma_start(out=xt[0:HALF, :], in_=xv[t, 0:HALF])
                nc.scalar.dma_start(out=xt[HALF:P, :], in_=xv[t, HALF:P])
                x3 = xt[:, :].rearrange("p (r d) -> p r d", d=D)
                nc.vector.tensor_mul(x3, x3, a_ap)
                yt = ypool.tile([P, TILE_FREE], bf16)
                y3 = yt[:, :].rearrange("p (r d) -> p r d", d=D)
                if GP_EVERY and t % GP_EVERY == (GP_EVERY - 1):
                    nc.gpsimd.tensor_add(y3, x3, b_ap)
                else:
                    nc.vector.tensor_add(y3, x3, b_ap)
                nc.gpsimd.dma_start(out=yv[t], in_=yt[:])
    nc.finalize()
    return nc


def _get_nc(rows_per_core: int):
    nc = _BUILD_CACHE.get(rows_per_core)
    if nc is None:
        nc = _build(rows_per_core)
        _BUILD_CACHE[rows_per_core] = nc
    return nc


# test.py reads this after a traced call for HW timing info.
LAST_RESULTS = None


def _bf16_to_f32(a: np.ndarray) -> np.ndarray:
    """Exact bf16 -> f32 upcast via bit manipulation (no ml_dtypes needed)."""
    u = np.asarray(a).view(np.uint16).astype(np.uint32) << 16
    return u.view(np.float32)


def kernel(
    x: np.ndarray,
    A_diag: np.ndarray,
    B: np.ndarray,
    trace: bool = False,
    **trace_kwargs,
) -> np.ndarray:
    from concourse.bass_utils import run_bass_kernel_spmd

    global LAST_RESULTS

    x = np.ascontiguousarray(np.asarray(x, dtype=np.float32))
    A_diag = np.asarray(A_diag, dtype=np.float32).reshape(D)
    B = np.asarray(B, dtype=np.float32).reshape(D)
    assert x.shape == (N, D)

    a_rep = np.ascontiguousarray(np.broadcast_to(A_diag, (P, D)))
    b_rep = np.ascontiguousarray(np.broadcast_to(B, (P, D)))

    in_maps = [
        {
            "x": x[i * ROWS_PER_CORE : (i + 1) * ROWS_PER_CORE],
            "a_rep": a_rep,
            "b_rep": b_rep,
        }
        for i in range(N_CORES)
    ]

    nc = _get_nc(ROWS_PER_CORE)
    res = run_bass_kernel_spmd(
        nc, in_maps, list(range(N_CORES)), trace=trace, **trace_kwargs
    )
    LAST_RESULTS = res
    out = np.concatenate([_bf16_to_f32(r["y"]) for r in res.results], axis=0)
    return np.ascontiguousarray(out)


if __name__ == "__main__":
    xs = np.random.randn(N, D).astype(np.float32)
    ad = np.random.randn(D).astype(np.float32)
    bs = np.random.randn(D).astype(np.float32)
    y = kernel(xs, ad, bs)
    ref = xs * ad + bs
    err = np.max(np.abs(y - ref) / np.maximum(np.abs(ref), 1e-6))
    print("max rel err:", err)


# revision 4
# speedup vs baseline: 1.0605x; 1.0605x over previous
"""DiagonalAffine kernel for Trainium2: y = x * A_diag + B.

x: (262144, 512) f32. Data-parallel over 8 NeuronCores: each core gets a
contiguous slice of 32768 rows.

Per-core design (derived from NTFF trace analysis of the f32 baseline):
the 16 SDMA engines aggregate ~424 GB/s one-way and were 95% busy, and the
DVE (0.96 GHz, 1 f32 elem/cycle on tensor_tensor) was 85% busy doing
2 ops/element. Two levers:

1. Store the output as bf16 (final rounding error <= 2^-8 relative to each
   element -- well inside the 2e-2 gate; the f32 mul/add path is bit-exact
   vs the reference). Traffic drops 128MiB -> 96MiB per core.
2. Balance the DMA queues so all three drain together: loads are split by
   partition half across the two HWDGE rings (sync: partitions 0-63 = even
   SDMA engines, scalar: 64-127 = odd engines), stores ride the SWDGE
   (gpsimd) queue with 8KB partition lines vs the loads' 16KB lines --
   each engine sees 2:1 load:store bytes per round-robin cycle, matching
   the 2:1 byte ratio of the streams.

Compute: DVE does every f32 multiply (bit-exactness: a mul deviation
scales with |x*a| and would blow the elementwise rel check at cancellation
points; an add deviation scales with |y| and is safe) plus half the adds;
gpsimd (2x slower per element) does the other half of the adds and the
store descriptor generation. Adds write bf16 tiles directly.
"""

import os
import sys

import numpy as np

_TRN_REPO = "/opt/trn_rl_repo"
if os.path.isdir(_TRN_REPO) and _TRN_REPO not in sys.path:
    sys.path.insert(0, _TRN_REPO)

N, D = 262144, 512
N_CORES = 8
ROWS_PER_CORE = N // N_CORES  # 32768

P = 128              # SBUF partitions
F_ROWS = int(os.environ.get("K_F_ROWS", "4"))   # rows of x per partition per tile
TILE_FREE = F_ROWS * D
ROWS_PER_TILE = P * F_ROWS                      # 512
X_BUFS = int(os.environ.get("K_XBUFS", "8"))
Y_BUFS = int(os.environ.get("K_YBUFS", "8"))
# which tiles' adds run on gpsimd: t % 8 in this set (24/64 tiles)
GP_SET = tuple(
    int(s) for s in os.environ.get("K_GP_SET", "1,4,6").split(",") if s != ""
)

_BUILD_CACHE: dict = {}


def _build(rows_per_core: int):
    """Build the per-core Bass program (identical on all cores)."""
    import concourse.bacc as bacc
    import concourse.tile as tile
    from concourse import mybir

    f32 = mybir.dt.float32
    bf16 = mybir.dt.bfloat16
    n_tiles = rows_per_core // ROWS_PER_TILE
    assert n_tiles * ROWS_PER_TILE == rows_per_core

    nc = bacc.Bacc("TRN2", debug=False, num_devices=N_CORES)
    x_in = nc.dram_tensor("x", [rows_per_core, D], f32, kind="ExternalInput")
    a_in = nc.dram_tensor("a_rep", [P, D], f32, kind="ExternalInput")
    b_in = nc.dram_tensor("b_rep", [P, D], f32, kind="ExternalInput")
    y_out = nc.dram_tensor("y", [rows_per_core, D], bf16, kind="ExternalOutput")

    xv = x_in[:, :].rearrange("(t p f) d -> t p (f d)", p=P, f=F_ROWS)
    yv = y_out[:, :].rearrange("(t p f) d -> t p (f d)", p=P, f=F_ROWS)
    HALF = P // 2

    with tile.TileContext(nc) as tc:
        with (
            tc.tile_pool(name="const", bufs=1) as cpool,
            tc.tile_pool(name="xp", bufs=X_BUFS) as xpool,
            tc.tile_pool(name="yp", bufs=Y_BUFS) as ypool,
        ):
            a_t = cpool.tile([P, D], f32, tag="a")
            nc.sync.dma_start(out=a_t[:], in_=a_in[:, :])
            b_t = cpool.tile([P, D], f32, tag="b")
            nc.scalar.dma_start(out=b_t[:], in_=b_in[:, :])

            a_ap = a_t[:, :].unsqueeze(1).to_broadcast((P, F_ROWS, D))
            b_ap = b_t[:, :].unsqueeze(1).to_broadcast((P, F_ROWS, D))

            for t in range(n_tiles):
                xt = xpool.tile([P, TILE_FREE], f32)
                if t % 2 == 0:
                    nc.sync.dma_start(out=xt[:], in_=xv[t])
                else:
                    nc.scalar.dma_start(out=xt[:], in_=xv[t])
                x3 = xt[:, :].rearrange("p (r d) -> p r d", d=D)
                nc.vector.tensor_mul(x3, x3, a_ap)
                # f32 in-place add (mixed-dtype TT is ~3x slower on DVE and
                # gpsimd, so the bf16 cast runs on the idle ACT engine).
                if t % 8 in GP_SET:
                    nc.gpsimd.tensor_add(x3, x3, b_ap)
                else:
                    nc.vector.tensor_add(x3, x3, b_ap)
                yt = ypool.tile([P, TILE_FREE], bf16)
                nc.scalar.copy(out=yt[:], in_=xt[:])
                nc.gpsimd.dma_start(out=yv[t], in_=yt[:])
    nc.finalize()
    return nc


def _get_nc(rows_per_core: int):
    nc = _BUILD_CACHE.get(rows_per_core)
    if nc is None:
        nc = _build(rows_per_core)
        _BUILD_CACHE[rows_per_core] = nc
    return nc


# test.py reads this after a traced call for HW timing info.
LAST_RESULTS = None


def _bf16_to_f32(a: np.ndarray) -> np.ndarray:
    """Exact bf16 -> f32 upcast via bit manipulation (no ml_dtypes needed)."""
    u = np.asarray(a).view(np.uint16).astype(np.uint32) << 16
    return u.view(np.float32)


def kernel(
    x: np.ndarray,
    A_diag: np.ndarray,
    B: np.ndarray,
    trace: bool = False,
    **trace_kwargs,
) -> np.ndarray:
    from concourse.bass_utils import run_bass_kernel_spmd

    global LAST_RESULTS

    x = np.ascontiguousarray(np.asarray(x, dtype=np.float32))
    A_diag = np.asarray(A_diag, dtype=np.float32).reshape(D)
    B = np.asarray(B, dtype=np.float32).reshape(D)
    assert x.shape == (N, D)

    a_rep = np.ascontiguousarray(np.broadcast_to(A_diag, (P, D)))
    b_rep = np.ascontiguousarray(np.broadcast_to(B, (P, D)))

    in_maps = [
        {
            "x": x[i * ROWS_PER_CORE : (i + 1) * ROWS_PER_CORE],
            "a_rep": a_rep,
            "b_rep": b_rep,
        }
        for i in range(N_CORES)
    ]

    nc = _get_nc(ROWS_PER_CORE)
    res = run_bass_kernel_spmd(
        nc, in_maps, list(range(N_CORES)), trace=trace, **trace_kwargs
    )
    LAST_RESULTS = res
    out = np.concatenate([_bf16_to_f32(r["y"]) for r in res.results], axis=0)
    return np.ascontiguousarray(out)


if __name__ == "__main__":
    xs = np.random.randn(N, D).astype(np.float32)
    ad = np.random.randn(D).astype(np.float32)
    bs = np.random.randn(D).astype(np.float32)
    y = kernel(xs, ad, bs)
    ref = xs * ad + bs
    err = np.max(np.abs(y - ref) / np.maximum(np.abs(ref), 1e-6))
    print("max rel err:", err)


# revision 5
# speedup vs baseline: 1.2696x; 1.1972x over previous
"""DiagonalAffine kernel for Trainium2: y = x * A_diag + B.

x: (262144, 512) f32. Data-parallel over 8 NeuronCores: each core gets a
contiguous slice of 32768 rows.

Per-core design (derived from NTFF trace analysis of the f32 baseline):
the 16 SDMA engines aggregate ~424 GB/s one-way and were 95% busy, and the
DVE (0.96 GHz, 1 f32 elem/cycle on tensor_tensor) was 85% busy doing
2 ops/element. Two levers:

1. Store the output as bf16 (final rounding error <= 2^-8 relative to each
   element -- well inside the 2e-2 gate; the f32 mul/add path is bit-exact
   vs the reference). Traffic drops 128MiB -> 96MiB per core.
2. Balance the DMA queues so all three drain together: loads are split by
   partition half across the two HWDGE rings (sync: partitions 0-63 = even
   SDMA engines, scalar: 64-127 = odd engines), stores ride the SWDGE
   (gpsimd) queue with 8KB partition lines vs the loads' 16KB lines --
   each engine sees 2:1 load:store bytes per round-robin cycle, matching
   the 2:1 byte ratio of the streams.

Compute: DVE does every f32 multiply (bit-exactness: a mul deviation
scales with |x*a| and would blow the elementwise rel check at cancellation
points; an add deviation scales with |y| and is safe) plus half the adds;
gpsimd (2x slower per element) does the other half of the adds and the
store descriptor generation. Adds write bf16 tiles directly.
"""

import os
import sys

import numpy as np

_TRN_REPO = "/opt/trn_rl_repo"
if os.path.isdir(_TRN_REPO) and _TRN_REPO not in sys.path:
    sys.path.insert(0, _TRN_REPO)

N, D = 262144, 512
N_CORES = 8
ROWS_PER_CORE = N // N_CORES  # 32768

P = 128              # SBUF partitions
F_ROWS = int(os.environ.get("K_F_ROWS", "4"))   # rows of x per partition per tile
TILE_FREE = F_ROWS * D
ROWS_PER_TILE = P * F_ROWS                      # 512
X_BUFS = int(os.environ.get("K_XBUFS", "8"))
Y_BUFS = int(os.environ.get("K_YBUFS", "8"))
# which tiles' adds run on gpsimd: t % 8 in this set (24/64 tiles)
GP_SET = tuple(
    int(s) for s in os.environ.get("K_GP_SET", "1,4,6").split(",") if s != ""
)

_BUILD_CACHE: dict = {}


def _build(rows_per_core: int):
    """Build the per-core Bass program (identical on all cores)."""
    import concourse.bacc as bacc
    import concourse.tile as tile
    from concourse import mybir

    f32 = mybir.dt.float32
    bf16 = mybir.dt.bfloat16
    n_tiles = rows_per_core // ROWS_PER_TILE
    assert n_tiles * ROWS_PER_TILE == rows_per_core

    nc = bacc.Bacc("TRN2", debug=False, num_devices=N_CORES)
    x_in = nc.dram_tensor("x", [rows_per_core, D], f32, kind="ExternalInput")
    a_in = nc.dram_tensor("a_rep", [P, D], f32, kind="ExternalInput")
    b_in = nc.dram_tensor("b_rep", [P, D], f32, kind="ExternalInput")
    y_out = nc.dram_tensor("y", [rows_per_core, D], bf16, kind="ExternalOutput")

    xv = x_in[:, :].rearrange("(t p f) d -> t p (f d)", p=P, f=F_ROWS)
    yv = y_out[:, :].rearrange("(t p f) d -> t p (f d)", p=P, f=F_ROWS)
    HALF = P // 2

    with tile.TileContext(nc) as tc:
        with (
            tc.tile_pool(name="const", bufs=1) as cpool,
            tc.tile_pool(name="xp", bufs=X_BUFS) as xpool,
            tc.tile_pool(name="yp", bufs=Y_BUFS) as ypool,
        ):
            a_t = cpool.tile([P, D], f32, tag="a")
            nc.sync.dma_start(out=a_t[:], in_=a_in[:, :])
            b_t = cpool.tile([P, D], f32, tag="b")
            nc.scalar.dma_start(out=b_t[:], in_=b_in[:, :])

            a_ap = a_t[:, :].unsqueeze(1).to_broadcast((P, F_ROWS, D))
            b_ap = b_t[:, :].unsqueeze(1).to_broadcast((P, F_ROWS, D))

            for t in range(n_tiles):
                xt = xpool.tile([P, TILE_FREE], f32)
                if t % 2 == 0:
                    nc.sync.dma_start(out=xt[:], in_=xv[t])
                else:
                    nc.scalar.dma_start(out=xt[:], in_=xv[t])
                x3 = xt[:, :].rearrange("p (r d) -> p r d", d=D)
                nc.vector.tensor_mul(x3, x3, a_ap)
                yt = ypool.tile([P, TILE_FREE], bf16)
                y3 = yt[:, :].rearrange("p (r d) -> p r d", d=D)
                # add writes the bf16 tile directly (no separate cast pass --
                # total SBUF traffic is the shared wall).
                if t % 8 in GP_SET:
                    nc.gpsimd.tensor_add(y3, x3, b_ap)
                else:
                    nc.vector.tensor_add(y3, x3, b_ap)
                nc.gpsimd.dma_start(out=yv[t], in_=yt[:])
    nc.finalize()
    return nc


def _get_nc(rows_per_core: int):
    nc = _BUILD_CACHE.get(rows_per_core)
    if nc is None:
        nc = _build(rows_per_core)
        _BUILD_CACHE[rows_per_core] = nc
    return nc


# test.py reads this after a traced call for HW timing info.
LAST_RESULTS = None


def _bf16_to_f32(a: np.ndarray) -> np.ndarray:
    """Exact bf16 -> f32 upcast via bit manipulation (no ml_dtypes needed)."""
    u = np.asarray(a).view(np.uint16).astype(np.uint32) << 16
    return u.view(np.float32)


def kernel(
    x: np.ndarray,
    A_diag: np.ndarray,
    B: np.ndarray,
    trace: bool = False,
    **trace_kwargs,
) -> np.ndarray:
    from concourse.bass_utils import run_bass_kernel_spmd

    global LAST_RESULTS

    x = np.ascontiguousarray(np.asarray(x, dtype=np.float32))
    A_diag = np.asarray(A_diag, dtype=np.float32).reshape(D)
    B = np.asarray(B, dtype=np.float32).reshape(D)
    assert x.shape == (N, D)

    a_rep = np.ascontiguousarray(np.broadcast_to(A_diag, (P, D)))
    b_rep = np.ascontiguousarray(np.broadcast_to(B, (P, D)))

    in_maps = [
        {
            "x": x[i * ROWS_PER_CORE : (i + 1) * ROWS_PER_CORE],
            "a_rep": a_rep,
            "b_rep": b_rep,
        }
        for i in range(N_CORES)
    ]

    nc = _get_nc(ROWS_PER_CORE)
    res = run_bass_kernel_spmd(
        nc, in_maps, list(range(N_CORES)), trace=trace, **trace_kwargs
    )
    LAST_RESULTS = res
    out = np.concatenate([_bf16_to_f32(r["y"]) for r in res.results], axis=0)
    return np.ascontiguousarray(out)


if __name__ == "__main__":
    xs = np.random.randn(N, D).astype(np.float32)
    ad = np.random.randn(D).astype(np.float32)
    bs = np.random.randn(D).astype(np.float32)
    y = kernel(xs, ad, bs)
    ref = xs * ad + bs
    err = np.max(np.abs(y - ref) / np.maximum(np.abs(ref), 1e-6))
    print("max rel err:", err)
